# revision 2
# baseline (speedup 1.0000x reference)
"""Multi-head attention (B=4, S=2048, D=1024, H=16) on 8 Trainium2 NeuronCores.

Sharding: core c -> batch c//2, head-group c%2 (8 heads = 512 dims each).
Each core computes qkv projection, softmax attention and its partial
out-projection (Megatron row-split of w_out); the host sums core pairs.

v2: single fused pipeline, all bf16 matmuls.
 - The ACT engine (exp, ~255us/core) and the PE (matmuls, ~275us) are
   kept continuously busy: qkv projection, v production and the out
   projection are interleaved into the attention loop as filler so the
   PE never idles (idle PE drops to the 1.2GHz p-state; a dense stream
   ramps to 2.4GHz).
 - scores matmuls for the two heads of a pair use PE row-halves
   (tile_position (0,0)/(64,0)) and execute concurrently.
 - x stays resident in SBUF (4MB bf16), read once from HBM.
 - softmax denominators come free from a 1.0-column appended to V;
   partition-broadcast of the reciprocal row via a DRAM round-trip DMA.
"""

import numpy as np
import ml_dtypes

B, S, D, H = 4, 2048, 1024, 16
HD = D // H          # 64
HG = H // 2          # 8 heads per core
DG = HG * HD         # 512 local head-cat dims
SCALE = HD ** -0.5   # folded into wq host-side
NCORES = 8
NDT = 8              # contraction tiles of 128
NSQT = 16            # sequence tiles of 128
NPAIR = 4            # head pairs
SQQ = 512            # q-chunk
NCI = S // SQQ       # 4 chunks of 512 tokens

_CACHE = {}


# --------------------------------------------------------------------------
# wait splitting: this toolchain's walrus rejects >1 sync wait per instruction
# on some paths; move excess semaphore waits onto same-engine NoOps.
# --------------------------------------------------------------------------
def _split_excess_waits(nc, max_waits=1):
    import bass_rust
    import concourse.mybir as mybir

    ctr = [0]
    for fn in nc.m.functions:
        for bb in fn.blocks:
            insts = list(bb.instructions)
            out = []
            changed = False
            for inst in insts:
                si = inst.sync_info
                waits = list(si.on_wait) if si is not None and si.on_wait else []
                sem_waits = [w for w in waits if w.sync_type == "semaphore"]
                other = [w for w in waits if w.sync_type != "semaphore"]
                budget = max_waits - len(other)
                if len(sem_waits) > budget and budget >= 1:
                    head, keep = sem_waits[:-budget], sem_waits[-budget:]
                    chunks = [
                        head[i : i + max_waits]
                        for i in range(0, len(head), max_waits)
                    ]
                    for ch in chunks:
                        nop = mybir.InstNoOp(
                            name=f"wsplit-{ctr[0]}",
                            opcode="NoOp",
                            engine=inst.engine,
                            ins=[],
                            outs=[],
                        )
                        nop.sync_info = bass_rust.SyncInfo(on_wait=ch, on_update=[])
                        ctr[0] += 1
                        out.append(nop)
                    inst.sync_info = bass_rust.SyncInfo(
                        on_wait=other + keep,
                        on_update=list(si.on_update) if si.on_update else [],
                    )
                    changed = True
                out.append(inst)
            if changed:
                bb.instructions = out


# --------------------------------------------------------------------------
# device program (identical on all 8 cores)
# --------------------------------------------------------------------------
def _build():
    import concourse.bass as bass
    import concourse.tile as tile
    import concourse.mybir as mybir

    F32 = mybir.dt.float32
    BF16 = mybir.dt.bfloat16
    EXP = mybir.ActivationFunctionType.Exp
    ts = bass.ts

    nc = bass.Bass()

    xT = nc.dram_tensor("xT", [D, S], BF16, kind="ExternalInput")
    wq = nc.dram_tensor("wq", [D, DG], BF16, kind="ExternalInput")
    wk = nc.dram_tensor("wk", [D, DG], BF16, kind="ExternalInput")
    wv = nc.dram_tensor("wv", [D, DG], BF16, kind="ExternalInput")
    bqk = nc.dram_tensor("bqk", [128, 8], F32, kind="ExternalInput")
    bvt = nc.dram_tensor("bvt", [128, DG], F32, kind="ExternalInput")
    wo = nc.dram_tensor("wo", [NPAIR, 128, D], BF16, kind="ExternalInput")
    bot = nc.dram_tensor("bot", [128, D], F32, kind="ExternalInput")
    outp = nc.dram_tensor("outp", [S, D], F32, kind="ExternalOutput")

    with tile.TileContext(nc) as tc:
        with (
            tc.tile_pool(name="consts", bufs=1) as cons,
            tc.tile_pool(name="xsp", bufs=1) as xsp,
            tc.tile_pool(name="wp", bufs=1) as wp,
            tc.tile_pool(name="qkp", bufs=1) as qkp,
            tc.tile_pool(name="vtp", bufs=1) as vtp,
            tc.tile_pool(name="otp", bufs=1) as otp,
            tc.tile_pool(name="ptp", bufs=4) as ptp,
            tc.tile_pool(name="pvsp", bufs=2) as pvsp,
            tc.tile_pool(name="rcp", bufs=2) as rcp,
            tc.tile_pool(name="bcp", bufs=2) as bcp,
            tc.tile_pool(name="yp", bufs=3) as yyp,
            tc.tile_pool(name="rsp", bufs=4, space="DRAM") as rsp,
            tc.tile_pool(name="scp", bufs=2, space="PSUM") as scp,
            tc.tile_pool(name="pvp", bufs=1, space="PSUM") as pvp,
            tc.tile_pool(name="utl", bufs=1, space="PSUM") as utl,
        ):
            # ---------------- constant / weight loads --------------------
            bqk_t = cons.tile([128, 8], F32)
            nc.sync.dma_start(bqk_t[:], bqk[:, :])
            wq_t = [wp.tile([128, DG], BF16, name=f"wq{d}", tag=f"wq{d}") for d in range(NDT)]
            wk_t = [wp.tile([128, DG], BF16, name=f"wk{d}", tag=f"wk{d}") for d in range(NDT)]
            wv_t = [wp.tile([128, DG], BF16, name=f"wv{d}", tag=f"wv{d}") for d in range(NDT)]
            for d in range(NDT):
                nc.sync.dma_start(wq_t[d][:], wq[ts(d, 128), :])
                nc.sync.dma_start(wk_t[d][:], wk[ts(d, 128), :])
                nc.sync.dma_start(wv_t[d][:], wv[ts(d, 128), :])
            # x resident, read once; ci-major so pair-0 projection starts asap
            xt = [[None] * NCI for _ in range(NDT)]
            for ci in range(NCI):
                for d in range(NDT):
                    xt[d][ci] = xsp.tile([128, SQQ], BF16, name=f"x{d}_{ci}", tag=f"x{d}_{ci}")
                    nc.sync.dma_start(xt[d][ci][:], xT[ts(d, 128), ts(ci, SQQ)])
            bvt_t = cons.tile([128, DG], F32)
            nc.sync.dma_start(bvt_t[:], bvt[:, :])
            wo_t = [wp.tile([128, D], BF16, name=f"wo{p}", tag=f"wo{p}") for p in range(NPAIR)]
            for p in range(NPAIR):
                nc.sync.dma_start(wo_t[p][:], wo[p, :, :])
            bot_t = cons.tile([128, D], F32)
            nc.sync.dma_start(bot_t[:], bot[:, :])

            # persistent activations
            qT = [qkp.tile([128, S], BF16, name=f"qT{p}", tag=f"qT{p}") for p in range(NPAIR)]
            kT = [qkp.tile([128, S], BF16, name=f"kT{p}", tag=f"kT{p}") for p in range(NPAIR)]
            # v tiles per s-tile per head-pair: [128, 130]; per head h: cols
            # h*65..h*65+63 hold v, col h*65+64 holds 1.0 (denominator trick)
            vt = [[vtp.tile([128, 130], BF16, name=f"v{s}_{p}", tag=f"v{s}_{p}")
                   for p in range(NPAIR)] for s in range(NSQT)]
            oT = [otp.tile([128, S], BF16, name=f"oT{p}", tag=f"oT{p}") for p in range(NPAIR)]

            # ones columns of v (written once, before v data lands)
            for s in range(NSQT):
                for p in range(NPAIR):
                    va = vt[s][p][:].rearrange("p (h e) -> p h e", e=65)
                    nc.vector.memset(va[:, :, 64:65], 1.0)

            # ---------------- filler units -------------------------------
            # qk projection for pair pr: units of (qk, ch(=2ci), d-pair):
            # two LDWs, four 512-row matmuls; stationary reuse across the
            # two ci chunks keeps LDWEIGHTS deduped by walrus ldw-opt.
            def qk_project_units(pr):
                groups = []  # (which, list-of-units); which: 'k0','k1','q0','q1'
                for which, wt, bcol in (("k", wk_t, 4 + pr), ("q", wq_t, pr)):
                    for ch in range(2):
                        units = []
                        state = {}

                        def u_mm(dp, wt=wt, ch=ch, state=state):
                            if dp == 0:
                                state["a"] = utl.tile([128, SQQ], F32, name="ga", tag="ua")
                                state["b"] = utl.tile([128, SQQ], F32, name="gb", tag="ub")
                            for d in (2 * dp, 2 * dp + 1):
                                for q2, ps in ((0, state["a"]), (1, state["b"])):
                                    nc.tensor.matmul(
                                        ps[:], wt[d][:, ts(pr, 128)],
                                        xt[d][2 * ch + q2][:],
                                        start=(d == 0), stop=(d == NDT - 1),
                                    )

                        def u_drain(which=which, ch=ch, state=state, bcol=bcol):
                            dst = (qT if which == "q" else kT)[pr]
                            for q2, ps in ((0, state["a"]), (1, state["b"])):
                                nc.vector.tensor_scalar_add(
                                    dst[:, ts(2 * ch + q2, SQQ)], ps[:],
                                    bqk_t[:, bcol : bcol + 1],
                                )

                        for dp in range(4):
                            units.append((lambda dp=dp, f=u_mm: f(dp)))
                        units.append(u_drain)
                        groups.append((f"{which}{ch}", units))
                return groups

            # v production for s-tile si: 3 units (4 mms, 4 mms, drain)
            def v_units(si):
                ci, sj = divmod(si, 4)
                state = {}
                tag = "ua" if si % 2 == 0 else "ub"

                def u_mm(half, state=state, ci=ci, sj=sj, tag=tag):
                    if half == 0:
                        state["ps"] = utl.tile([128, DG], F32, name="vps", tag=tag)
                    for d in range(4 * half, 4 * half + 4):
                        nc.tensor.matmul(
                            state["ps"][:], xt[d][ci][:, ts(sj, 128)], wv_t[d][:],
                            start=(d == 0), stop=(d == NDT - 1),
                        )

                def u_drain(si=si, state=state):
                    ps3 = state["ps"][:].rearrange("p (h e) -> p h e", e=HD)
                    bv3 = bvt_t[:].rearrange("p (h e) -> p h e", e=HD)
                    for p in range(NPAIR):
                        va = vt[si][p][:].rearrange("p (h e) -> p h e", e=65)
                        nc.vector.tensor_add(
                            va[:, :, 0:HD],
                            ps3[:, 2 * p : 2 * p + 2, :],
                            bv3[:, 2 * p : 2 * p + 2, :],
                        )

                return [lambda f=u_mm: f(0), lambda f=u_mm: f(1), u_drain]

            # out-projection for token chunk qu: 2 units per 128-token tile
            def outproj_units(qu):
                units = []
                for t4 in range(4):
                    tok = qu * SQQ + t4 * 128
                    state = {}

                    def u_mm(tok=tok, state=state):
                        pa = utl.tile([128, SQQ], F32, name="pa", tag="ua")
                        pb = utl.tile([128, SQQ], F32, name="pb", tag="ub")
                        for p in range(NPAIR):
                            for j2, ps in ((0, pa), (1, pb)):
                                nc.tensor.matmul(
                                    ps[:], oT[p][:, tok : tok + 128],
                                    wo_t[p][:, ts(j2, SQQ)],
                                    start=(p == 0), stop=(p == NPAIR - 1),
                                )
                        state["pa"], state["pb"] = pa, pb

                    def u_drain(tok=tok, state=state):
                        for j2, ps in ((0, state["pa"]), (1, state["pb"])):
                            y = yyp.tile([128, SQQ], F32, name="y", tag="y")
                            nc.vector.tensor_add(y[:], ps[:], bot_t[:, ts(j2, SQQ)])
                            nc.sync.dma_start(outp[tok : tok + 128, ts(j2, SQQ)], y[:])

                    units.append(u_mm)
                    units.append(u_drain)
                return units

            # ---------------- main schedule ------------------------------
            class Group:
                def __init__(self, units):
                    self.units = list(units)
                    self.i = 0

                def pop1(self):
                    if self.i < len(self.units):
                        self.units[self.i]()
                        self.i += 1
                        return True
                    return False

                def drain(self):
                    while self.pop1():
                        pass

            groups = []  # filler groups, priority order

            def backlog():
                return sum(len(gg.units) - gg.i for gg in groups)

            def pop_filler():
                n = 2 if backlog() > 24 else 1
                for _ in range(n):
                    for gg in groups:
                        if gg.pop1():
                            break

            def emit_pv(pr, pv, pt, s):
                if vg[s] is not None:
                    # v for this s-tile must be emitted before its consumer
                    vg[s].drain()
                    vg[s] = None
                for h in range(2):
                    va = vt[s][pr][:].rearrange("p (h e) -> p h e", e=65)
                    nc.tensor.matmul(
                        pv[:, ts(h, SQQ)], va[:, h, :], pt[:, ts(h, SQQ)],
                        start=(s == 0), stop=(s == NSQT - 1),
                    )

            # pair-0 projection lead-in: k for s-tiles 0-7 and q for qu0/qu1
            # inline; the rest rides as (force-drained) priority filler
            p0 = dict(qk_project_units(0))
            for name in ("k0", "q0"):
                for u in p0[name]:
                    u()
            k1g = Group(p0["k1"])
            groups.append(k1g)
            # v for the first s-tiles inline so pv never leads it
            vg = [Group(v_units(si)) for si in range(NSQT)]
            for si in range(2):
                vg[si].drain()
                vg[si] = None
            groups.extend(g for g in vg if g is not None)
            q1g = Group(p0["q1"])
            groups.append(q1g)
            qk_groups = {}
            for pr in range(1, NPAIR):
                qk_groups[pr] = [Group(units) for _, units in qk_project_units(pr)]

            for pr in range(NPAIR):
                if pr > 0:
                    # qT/kT for this pair must be complete before its scores
                    for gg in qk_groups[pr]:
                        gg.drain()
                if pr + 1 < NPAIR:
                    groups.extend(qk_groups[pr + 1])
                for qu in range(NCI):
                    if pr == 0 and qu == 2:
                        q1g.drain()
                    qs = ts(qu, SQQ)
                    pv = pvp.tile([65, 1024], F32, name="pv", tag="pv")
                    pts = {}
                    for s in range(NSQT):
                        if pr == 0 and qu == 0 and s == 7:
                            k1g.drain()
                        sc = scp.tile([128, 1024], F32, name="sc", tag="sc")
                        nc.tensor.matmul(
                            sc[:, 0:SQQ], kT[pr][0:HD, ts(s, 128)], qT[pr][0:HD, qs],
                            start=True, stop=True,
                        )
                        nc.tensor.matmul(
                            sc[:, SQQ:1024], kT[pr][HD:128, ts(s, 128)], qT[pr][HD:128, qs],
                            start=True, stop=True,
                        )
                        pt = ptp.tile([128, 1024], BF16, name="pt", tag="pt")
                        nc.scalar.activation(pt[:], sc[:], EXP)
                        pts[s] = pt
                        # pv trails scores by 2 steps so the PE never
                        # head-of-line blocks on the ACT result
                        if s >= 3:
                            emit_pv(pr, pv, pts.pop(s - 3), s - 3)
                        pop_filler()
                    for st in (NSQT - 3, NSQT - 2, NSQT - 1):
                        emit_pv(pr, pv, pts.pop(st), st)
                    # copy pv psum -> sbuf immediately so the psum banks
                    # free up and normalization runs off the PE critical path
                    pvs = pvsp.tile([65, 1024], F32, name="pvs", tag="pvs")
                    nc.vector.tensor_copy(pvs[:], pv[:])
                    # normalization: the denom row round-trips through DRAM
                    # reshaped to [32, 32] so the reciprocal runs on 32 lanes
                    # (a [1,1024] reciprocal is ~8us on one lane and blocks
                    # the DVE queue); then broadcast back over 64 partitions
                    rs = rsp.tile([1, 1024], F32, name="rs", tag="rs")
                    nc.sync.dma_start(rs[:], pvs[64:65, :])
                    den = rcp.tile([32, 32], F32, name="den", tag="den")
                    nc.sync.dma_start(
                        den[:], rs[:].rearrange("o (p f) -> (o p) f", f=32))
                    rc = rcp.tile([32, 32], F32, name="rc", tag="rc")
                    nc.vector.reciprocal(rc[:], den[:])
                    rs2 = rsp.tile([1, 1024], F32, name="rs2", tag="rs2")
                    nc.sync.dma_start(
                        rs2[:].rearrange("o (p f) -> (o p) f", f=32), rc[:])
                    bc = bcp.tile([HD, 1024], F32, name="bc", tag="bc")
                    nc.sync.dma_start(bc[:], rs2[:].broadcast_to([HD, 1024]))
                    nc.vector.tensor_mul(
                        oT[pr][0:HD, qs], pvs[0:HD, 0:SQQ], bc[:, 0:SQQ]
                    )
                    nc.vector.tensor_mul(
                        oT[pr][HD:128, qs], pvs[0:HD, SQQ:1024], bc[:, SQQ:1024]
                    )
                    if pr == NPAIR - 1:
                        groups.append(Group(outproj_units(qu)))
            while any(g.i < len(g.units) for g in groups):
                pop_filler()

    _split_excess_waits(nc, max_waits=1)
    return nc


def _get_nc():
    if "nc" not in _CACHE:
        _CACHE["nc"] = _build()
    return _CACHE["nc"]


# --------------------------------------------------------------------------
# host entry point
# --------------------------------------------------------------------------
def _shard_inputs(x, w_qkv, b_qkv, w_out, b_out):
    f = np.float32
    BF = ml_dtypes.bfloat16
    x = np.asarray(x, f)
    w_qkv = np.asarray(w_qkv, f)
    b_qkv = np.asarray(b_qkv, f)
    w_out = np.asarray(w_out, f)
    b_out = np.asarray(b_out, f)
    in_maps = []
    for c in range(NCORES):
        b, g = divmod(c, 2)
        cols = slice(DG * g, DG * (g + 1))
        wq_c = (w_qkv[:, 0:D][:, cols] * np.float32(SCALE)).astype(BF)
        wk_c = w_qkv[:, D : 2 * D][:, cols].astype(BF)
        wv_c = w_qkv[:, 2 * D :][:, cols].astype(BF)
        bq_c = (b_qkv[0:D][cols] * np.float32(SCALE)).reshape(4, 128).T
        bk_c = b_qkv[D : 2 * D][cols].reshape(4, 128).T
        bqk_c = np.ascontiguousarray(np.concatenate([bq_c, bk_c], axis=1), f)
        bv_c = np.ascontiguousarray(np.tile(b_qkv[2 * D :][cols], (128, 1)), f)
        wo_c = np.ascontiguousarray(
            w_out[DG * g : DG * (g + 1), :].reshape(NPAIR, 128, D)
        ).astype(BF)
        bo_c = (
            np.ascontiguousarray(np.tile(b_out, (128, 1)), f)
            if g == 0
            else np.zeros((128, D), f)
        )
        in_maps.append(
            {
                "xT": np.ascontiguousarray(x[b].T).astype(BF),
                "wq": np.ascontiguousarray(wq_c),
                "wk": np.ascontiguousarray(wk_c),
                "wv": np.ascontiguousarray(wv_c),
                "bqk": bqk_c,
                "bvt": bv_c,
                "wo": wo_c,
                "bot": bo_c,
            }
        )
    return in_maps


def _patch_ldw_opt():
    """Flip walrus --enable-ldw-opt to true (dedupe repeated LDWEIGHTS for
    consecutive same-stationary matmuls). Controlled by KERNEL_LDW_OPT env."""
    import os
    # walrus ldw-opt rejects bf16 Ldweights ("not compatible with LDW
    # optimization"), so it stays off for this all-bf16 kernel.
    if os.environ.get("KERNEL_LDW_OPT", "0") != "1":
        return
    if _CACHE.get("ldw_patched"):
        return
    import concourse.bass_utils as bu

    orig = bu.run_command

    def run_command_ldw(argv, **kwargs):
        argv = [a.replace("--enable-ldw-opt=false", "--enable-ldw-opt=true")
                if isinstance(a, str) else a for a in argv]
        return orig(argv, **kwargs)

    bu.run_command = run_command_ldw
    _CACHE["ldw_patched"] = True


def kernel(x, w_qkv, b_qkv, w_out, b_out, _trace=False, _trace_kwargs=None):
    from concourse.bass_utils import run_bass_kernel_spmd

    _patch_ldw_opt()
    nc = _get_nc()
    in_maps = _shard_inputs(x, w_qkv, b_qkv, w_out, b_out)
    kw = {}
    if _trace:
        kw["trace"] = True
        kw.update(_trace_kwargs or {})
    res = run_bass_kernel_spmd(nc, in_maps, core_ids=list(range(NCORES)), **kw)
    _CACHE["last_result"] = res
    parts = [r["outp"] for r in res.results]
    out = np.stack([parts[2 * b] + parts[2 * b + 1] for b in range(B)])
    return np.ascontiguousarray(out, np.float32)
